# revision 9
# baseline (speedup 1.0000x reference)
"""Dense Synthesizer Attention — Trainium2 Bass kernel.

Sharding: data-parallel over batch. B=8 batch elements, 8 NeuronCores,
one batch element per core, zero collectives.

Per-core computation (S=1024 tokens, F=512 feat, H=8 heads, dk=64):
    hT  = relu(w1^T @ qT + b1)          [1024, 1024]   (qT via PE transpose)
    awT = w2^T @ hT + b2                [512, 1024]
    per head h: aw_hT = awT[64h:64h+64, :]
      scores_m = aw_hT[:, m-tile].T @ aw_hT    (K=64, fp32r)
      E = exp(scores/8)  (bf16, ScalarE, accum_out -> row sums r)
      y_m = sum_c E[c-chunk, m-tile].T @ v[c-chunk, head]   (bf16)
            -- uses E == E^T (scores symmetric), so the same E tiles
               serve as both attn rows and attn^T columns.
      x_m = y_m * (1/r)   (per-partition scale), PE-transpose into xT
    out = xT.T @ wo + bo                [1024, 512]    (fp32r)

All dims are multiples of 128; everything stays on-chip between stages.
"""

import math
from contextlib import ExitStack

import numpy as np

B, S, F = 8, 1024, 512
H, DK = 8, 64
HID = 2 * F
P = 128

N_CORES = 8

_CACHED_NC = None


def _build_nc():
    import concourse.bass as bass  # noqa: F401
    import concourse.mybir as mybir
    import concourse.tile as tile
    from concourse import bacc

    dt = mybir.dt
    AF = mybir.ActivationFunctionType
    f32, f32r, bf16 = dt.float32, dt.float32r, dt.bfloat16

    SC = S // P      # 8 token chunks
    FC = F // P      # 4 feature chunks
    KC = HID // P    # 8 hidden chunks
    NS = S // 512    # 2 moving-operand chunks over tokens

    nc = bacc.Bacc(
        "TRN2",
        target_bir_lowering=False,
        debug=False,
        num_devices=N_CORES,
    )

    q_d = nc.declare_dram_parameter("q", [S, F], f32, isOutput=False)
    v_d = nc.declare_dram_parameter("v", [S, F], f32, isOutput=False)
    w1_d = nc.declare_dram_parameter("w1", [F, HID], f32, isOutput=False)
    w2_d = nc.declare_dram_parameter("w2", [HID, F], f32, isOutput=False)
    wv_d = nc.declare_dram_parameter("wv", [F, F], f32, isOutput=False)
    wo_d = nc.declare_dram_parameter("wo", [F, F], f32, isOutput=False)
    b1_d = nc.declare_dram_parameter("b1r", [P, KC], f32, isOutput=False)
    b2_d = nc.declare_dram_parameter("b2r", [P, FC], f32, isOutput=False)
    bv_d = nc.declare_dram_parameter("bvb", [P, F], f32, isOutput=False)
    bo_d = nc.declare_dram_parameter("bob", [P, F], f32, isOutput=False)
    id_d = nc.declare_dram_parameter("ident", [P, P], f32, isOutput=False)
    out_d = nc.declare_dram_parameter("out", [S, F], f32, isOutput=True)

    with ExitStack() as ctx:
        tc = ctx.enter_context(tile.TileContext(nc))

        const = ctx.enter_context(tc.tile_pool(name="const", bufs=1))
        ld = ctx.enter_context(tc.tile_pool(name="ld", bufs=3))
        big = ctx.enter_context(tc.tile_pool(name="big", bufs=1))
        epool = ctx.enter_context(tc.tile_pool(name="epool", bufs=2))
        rpool = ctx.enter_context(tc.tile_pool(name="rpool", bufs=2))
        ypool = ctx.enter_context(tc.tile_pool(name="ypool", bufs=3))
        opool = ctx.enter_context(tc.tile_pool(name="opool", bufs=3))

        ps512 = ctx.enter_context(tc.tile_pool(name="ps512", bufs=2, space="PSUM"))
        ps_sc = ctx.enter_context(tc.tile_pool(name="ps_sc", bufs=2, space="PSUM"))
        ps_yt = ctx.enter_context(tc.tile_pool(name="ps_yt", bufs=2, space="PSUM"))

        # ---- constants ----
        ident = const.tile([P, P], f32r)
        nc.sync.dma_start(ident, id_d[:, :].bitcast(f32r))
        identr = ident

        w1sb = const.tile([P, FC, HID], f32r)
        nc.sync.dma_start(w1sb, w1_d.rearrange("(c p) k -> p c k", p=P).bitcast(f32r))
        w2sb = const.tile([P, KC, F], f32r)
        nc.sync.dma_start(w2sb, w2_d.rearrange("(c p) f -> p c f", p=P).bitcast(f32r))
        wvsb = const.tile([P, FC, F], f32r)
        nc.sync.dma_start(wvsb, wv_d.rearrange("(c p) f -> p c f", p=P).bitcast(f32r))
        wosb = const.tile([P, FC, F], f32r)
        nc.sync.dma_start(wosb, wo_d.rearrange("(c p) f -> p c f", p=P).bitcast(f32r))
        b1sb = const.tile([P, KC], f32)
        nc.sync.dma_start(b1sb, b1_d[:, :])
        b2sb = const.tile([P, FC], f32)
        nc.sync.dma_start(b2sb, b2_d[:, :])
        bvsb = const.tile([P, F], f32)
        nc.sync.dma_start(bvsb, bv_d[:, :])
        bosb = const.tile([P, F], f32)
        nc.sync.dma_start(bosb, bo_d[:, :])

        # ---- qT / valT: PE-transpose inputs into [F, S] layout ----
        qTsb = big.tile([P, FC, S], f32r, tag="qx")
        valTsb = big.tile([P, FC, S], f32r, tag="valT")
        for src, dstT in ((q_d, qTsb), (v_d, valTsb)):
            for sc in range(SC):
                t_in = ld.tile([P, F], f32r, tag="ld")
                nc.sync.dma_start(t_in, src[sc * P:(sc + 1) * P, :].bitcast(f32r))
                for fc in range(FC):
                    t_p = ps_yt.tile([P, P], f32, tag="pt")
                    nc.tensor.transpose(
                        t_p.bitcast(f32r),
                        t_in[:, fc * P:(fc + 1) * P],
                        identr,
                    )
                    nc.vector.tensor_copy(dstT[:, fc, sc * P:(sc + 1) * P], t_p)

        # ---- mlp1: hT = relu(w1^T @ qT + b1)  [HID, S] ----
        hTsb = big.tile([P, KC, S], f32r, tag="hT")
        for m in range(KC):
            for n in range(NS):
                h_p = ps512.tile([P, 512], f32, tag="ps")
                for c in range(FC):
                    nc.tensor.matmul(
                        h_p,
                        w1sb[:, c, m * P:(m + 1) * P],
                        qTsb[:, c, n * 512:(n + 1) * 512],
                        start=(c == 0),
                        stop=(c == FC - 1),
                    )
                nc.scalar.activation(
                    hTsb[:, m, n * 512:(n + 1) * 512], h_p, AF.Relu,
                    bias=b1sb[:, m:m + 1], scale=1.0,
                )

        # ---- mlp2: awT = w2^T @ hT + b2  [F, S] ----
        awTsb = big.tile([P, FC, S], f32r, tag="awT")
        for m in range(FC):
            for n in range(NS):
                a_p = ps512.tile([P, 512], f32, tag="ps")
                for c in range(KC):
                    nc.tensor.matmul(
                        a_p,
                        w2sb[:, c, m * P:(m + 1) * P],
                        hTsb[:, c, n * 512:(n + 1) * 512],
                        start=(c == 0),
                        stop=(c == KC - 1),
                    )
                nc.scalar.activation(
                    awTsb[:, m, n * 512:(n + 1) * 512], a_p, AF.Identity,
                    bias=b2sb[:, m:m + 1], scale=1.0,
                )

        # ---- v projection: v = value @ wv + bv  [S, F] natural, bf16 ----
        vsb = big.tile([P, SC, F], bf16, tag="v")
        for m in range(SC):
            v_p = ps512.tile([P, 512], f32, tag="ps")
            for c in range(FC):
                nc.tensor.matmul(
                    v_p,
                    valTsb[:, c, m * P:(m + 1) * P],
                    wvsb[:, c, :],
                    start=(c == 0),
                    stop=(c == FC - 1),
                )
            nc.vector.tensor_add(vsb[:, m, :], v_p, bvsb)

        # ---- per-head: scores -> exp -> attn@V -> xT ----
        xTsb = big.tile([P, FC, S], f32r, tag="qx")
        scale = 1.0 / math.sqrt(DK)

        e_tiles = [None] * H
        rinv_tiles = [None] * H

        def scores_exp(h):
            fc, po = h // 2, (h % 2) * DK
            aw_hT = awTsb[po:po + DK, fc, :]
            e_sb = epool.tile([P, SC, S], bf16, tag="E")
            rsum = rpool.tile([P, SC], f32, tag="rs")
            rinv = rpool.tile([P, SC], f32, tag="ri")
            e_tiles[h], rinv_tiles[h] = e_sb, rinv
            for m in range(SC):
                sc_p = ps_sc.tile([P, S], f32, tag="sc")
                for n in range(NS):
                    nc.tensor.matmul(
                        sc_p[:, n * 512:(n + 1) * 512],
                        aw_hT[:, m * P:(m + 1) * P],
                        aw_hT[:, n * 512:(n + 1) * 512],
                        start=True,
                        stop=True,
                    )
                nc.scalar.activation(
                    e_sb[:, m, :], sc_p, AF.Exp, scale=scale,
                    accum_out=rsum[:, m:m + 1],
                )
                nc.vector.reciprocal(rinv[:, m:m + 1], rsum[:, m:m + 1])

        def attn_v(h):
            fc, po = h // 2, (h % 2) * DK
            e_sb, rinv = e_tiles[h], rinv_tiles[h]
            for m in range(SC):
                y_p = ps_yt.tile([P, P], f32, tag="pt")
                for c in range(SC):
                    nc.tensor.matmul(
                        y_p[:, :DK],
                        e_sb[:, c, m * P:(m + 1) * P],
                        vsb[:, c, h * DK:(h + 1) * DK],
                        start=(c == 0),
                        stop=(c == SC - 1),
                    )
                y_sb = ypool.tile([P, DK], f32r, tag="y")
                nc.vector.tensor_scalar_mul(y_sb, y_p[:, :DK], rinv[:, m:m + 1])
                yt_p = ps_yt.tile([P, P], f32, tag="pt")
                nc.tensor.transpose(yt_p[:DK, :].bitcast(f32r), y_sb, identr)
                nc.scalar.copy(
                    xTsb[po:po + DK, fc, m * P:(m + 1) * P], yt_p[:DK, :]
                )

        # software-pipeline heads: scores(h+1) issued before attn_v(h)
        scores_exp(0)
        for h in range(1, H):
            scores_exp(h)
            attn_v(h - 1)
        attn_v(H - 1)

        # ---- final: out = x @ wo + bo  [S, F] ----
        for m in range(SC):
            o_p = ps512.tile([P, 512], f32, tag="ps")
            for c in range(FC):
                nc.tensor.matmul(
                    o_p,
                    xTsb[:, c, m * P:(m + 1) * P],
                    wosb[:, c, :],
                    start=(c == 0),
                    stop=(c == FC - 1),
                )
            o_sb = opool.tile([P, F], f32, tag="o")
            nc.vector.tensor_add(o_sb, o_p, bosb)
            nc.sync.dma_start(out_d[m * P:(m + 1) * P, :], o_sb)

    nc.compile()
    return nc


def _get_nc():
    global _CACHED_NC
    if _CACHED_NC is None:
        _CACHED_NC = _build_nc()
    return _CACHED_NC


def _make_in_maps(inputs):
    query = np.asarray(inputs["query"], np.float32)
    value = np.asarray(inputs["value"], np.float32)
    w1 = np.asarray(inputs["w1"], np.float32)
    b1 = np.asarray(inputs["b1"], np.float32)
    w2 = np.asarray(inputs["w2"], np.float32)
    b2 = np.asarray(inputs["b2"], np.float32)
    wv = np.asarray(inputs["wv"], np.float32)
    bv = np.asarray(inputs["bv"], np.float32)
    wo = np.asarray(inputs["wo"], np.float32)
    bo = np.asarray(inputs["bo"], np.float32)

    b1r = np.ascontiguousarray(b1.reshape(HID // P, P).T)
    b2r = np.ascontiguousarray(b2.reshape(F // P, P).T)
    bvb = np.ascontiguousarray(np.broadcast_to(bv, (P, F)))
    bob = np.ascontiguousarray(np.broadcast_to(bo, (P, F)))

    shared = dict(w1=w1, w2=w2, wv=wv, wo=wo, b1r=b1r, b2r=b2r, bvb=bvb, bob=bob,
                  ident=np.eye(P, dtype=np.float32))
    return [dict(q=query[i], v=value[i], **shared) for i in range(N_CORES)]


def kernel(**inputs):
    in_maps = _make_in_maps(inputs)

    from concourse.bass_utils import run_bass_kernel_spmd

    nc = _get_nc()
    res = run_bass_kernel_spmd(nc, in_maps, core_ids=list(range(N_CORES)))
    out = np.stack([res.results[i]["out"] for i in range(N_CORES)], axis=0)
    return out.astype(np.float32)


if __name__ == "__main__":
    nc = _get_nc()
    print("built ok")


# revision 12
# speedup vs baseline: 6210.1444x; 6210.1444x over previous
"""Dense Synthesizer Attention — Trainium2 Bass kernel.

Sharding: data-parallel over batch. B=8 batch elements, 8 NeuronCores,
one batch element per core, zero collectives.

Per-core computation (S=1024 tokens, F=512 feat, H=8 heads, dk=64):
    hT  = relu(w1^T @ qT + b1)          [1024, 1024]   (qT via PE transpose)
    awT = w2^T @ hT + b2                [512, 1024]
    per head h: aw_hT = awT[64h:64h+64, :]
      scores_m = aw_hT[:, m-tile].T @ aw_hT    (K=64, fp32r)
      E = exp(scores/8)  (bf16, ScalarE, accum_out -> row sums r)
      y_m = sum_c E[c-chunk, m-tile].T @ v[c-chunk, head]   (bf16)
            -- uses E == E^T (scores symmetric), so the same E tiles
               serve as both attn rows and attn^T columns.
      x_m = y_m * (1/r)   (per-partition scale), PE-transpose into xT
    out = xT.T @ wo + bo                [1024, 512]    (fp32r)

All dims are multiples of 128; everything stays on-chip between stages.
"""

import math
from contextlib import ExitStack

import numpy as np

B, S, F = 8, 1024, 512
H, DK = 8, 64
HID = 2 * F
P = 128

N_CORES = 8

_CACHED_NC = None


def _build_nc(repeat=1):
    import concourse.bass as bass  # noqa: F401
    import concourse.mybir as mybir
    import concourse.tile as tile
    from concourse import bacc

    dt = mybir.dt
    AF = mybir.ActivationFunctionType
    f32, f32r, bf16 = dt.float32, dt.float32r, dt.bfloat16

    SC = S // P      # 8 token chunks
    FC = F // P      # 4 feature chunks
    KC = HID // P    # 8 hidden chunks
    NS = S // 512    # 2 moving-operand chunks over tokens

    nc = bacc.Bacc(
        "TRN2",
        target_bir_lowering=False,
        debug=False,
        num_devices=N_CORES,
    )

    q_d = nc.declare_dram_parameter("q", [S, F], f32, isOutput=False)
    v_d = nc.declare_dram_parameter("v", [S, F], f32, isOutput=False)
    w1_d = nc.declare_dram_parameter("w1", [F, HID], f32, isOutput=False)
    w2_d = nc.declare_dram_parameter("w2", [HID, F], f32, isOutput=False)
    wv_d = nc.declare_dram_parameter("wv", [F, H * (DK + 1)], f32, isOutput=False)
    wo_d = nc.declare_dram_parameter("wo", [F, F], f32, isOutput=False)
    b1_d = nc.declare_dram_parameter("b1r", [P, KC], f32, isOutput=False)
    b2_d = nc.declare_dram_parameter("b2r", [P, FC], f32, isOutput=False)
    bv_d = nc.declare_dram_parameter("bvb", [P, H * (DK + 1)], f32, isOutput=False)
    bo_d = nc.declare_dram_parameter("bob", [P, F], f32, isOutput=False)
    id_d = nc.declare_dram_parameter("ident", [P, P], f32, isOutput=False)
    on_d = nc.declare_dram_parameter("ones64", [1, DK], f32, isOutput=False)
    out_d = nc.declare_dram_parameter("out", [S, F], f32, isOutput=True)

    with ExitStack() as ctx:
        tc = ctx.enter_context(tile.TileContext(nc))

        const = ctx.enter_context(tc.tile_pool(name="const", bufs=1))
        ld = ctx.enter_context(tc.tile_pool(name="ld", bufs=3))
        big = ctx.enter_context(tc.tile_pool(name="big", bufs=1))
        epool = ctx.enter_context(tc.tile_pool(name="epool", bufs=2))
        rpool = ctx.enter_context(tc.tile_pool(name="rpool", bufs=2))
        ypool = ctx.enter_context(tc.tile_pool(name="ypool", bufs=3))
        opool = ctx.enter_context(tc.tile_pool(name="opool", bufs=3))

        ps512 = ctx.enter_context(tc.tile_pool(name="ps512", bufs=2, space="PSUM"))
        ps_sc = ctx.enter_context(tc.tile_pool(name="ps_sc", bufs=2, space="PSUM"))
        ps_yt = ctx.enter_context(tc.tile_pool(name="ps_yt", bufs=2, space="PSUM"))

        # ---- constants ----
        ident = const.tile([P, P], f32r)
        nc.sync.dma_start(ident, id_d[:, :].bitcast(f32r))
        identr = ident
        ones64 = const.tile([1, DK], f32r)
        nc.sync.dma_start(ones64, on_d[:, :].bitcast(f32r))

        w1sb = const.tile([P, FC, HID], f32r)
        nc.sync.dma_start(w1sb, w1_d.rearrange("(c p) k -> p c k", p=P).bitcast(f32r))
        w2sb = const.tile([P, KC, F], f32r)
        nc.sync.dma_start(w2sb, w2_d.rearrange("(c p) f -> p c f", p=P).bitcast(f32r))
        wvsb = const.tile([P, FC, H * (DK + 1)], f32r)
        nc.sync.dma_start(wvsb, wv_d.rearrange("(c p) f -> p c f", p=P).bitcast(f32r))
        wosb = const.tile([P, FC, F], f32r)
        nc.sync.dma_start(wosb, wo_d.rearrange("(c p) f -> p c f", p=P).bitcast(f32r))
        b1sb = const.tile([P, KC], f32)
        nc.sync.dma_start(b1sb, b1_d[:, :])
        b2sb = const.tile([P, FC], f32)
        nc.sync.dma_start(b2sb, b2_d[:, :])
        bvsb = const.tile([P, H * (DK + 1)], f32)
        nc.sync.dma_start(bvsb, bv_d[:, :])
        bosb = const.tile([P, F], f32)
        nc.sync.dma_start(bosb, bo_d[:, :])

        # ---- compute body (repeated `repeat` times for perf measurement;
        # pool-tile reuse serializes iterations) ----
        for _rep in range(repeat):
            _build_body(
                nc, tc, mybir, ld, big, epool, rpool, ypool, opool,
                ps512, ps_sc, ps_yt,
                q_d, v_d, out_d,
                w1sb, w2sb, wvsb, wosb, b1sb, b2sb, bvsb, bosb, identr, ones64,
            )

    nc.compile()
    return nc


def _build_body(nc, tc, mybir, ld, big, epool, rpool, ypool, opool,
                ps512, ps_sc, ps_yt, q_d, v_d, out_d,
                w1sb, w2sb, wvsb, wosb, b1sb, b2sb, bvsb, bosb, identr, ones64):
    import math
    dt = mybir.dt
    AF = mybir.ActivationFunctionType
    f32, f32r, bf16 = dt.float32, dt.float32r, dt.bfloat16
    SC = S // P
    FC = F // P
    KC = HID // P
    NS = S // 512
    if True:
        # ---- qT / valT: PE-transpose inputs into [F, S] layout ----
        qTsb = big.tile([P, FC, S], f32r, tag="qx")
        valTsb = big.tile([P, FC, S], f32r, tag="valT")
        for src, dstT in ((q_d, qTsb), (v_d, valTsb)):
            for sc in range(SC):
                t_in = ld.tile([P, F], f32r, tag="ld")
                nc.sync.dma_start(t_in, src[sc * P:(sc + 1) * P, :].bitcast(f32r))
                for fc in range(FC):
                    t_p = ps_yt.tile([P, P], f32, tag="pt")
                    nc.tensor.transpose(
                        t_p.bitcast(f32r),
                        t_in[:, fc * P:(fc + 1) * P],
                        identr,
                    )
                    nc.vector.tensor_copy(dstT[:, fc, sc * P:(sc + 1) * P], t_p)

        # ---- mlp1: hT = relu(w1^T @ qT + b1)  [HID, S] ----
        hTsb = big.tile([P, KC, S], f32r, tag="hT")
        for m in range(KC):
            for n in range(NS):
                h_p = ps512.tile([P, 512], f32, tag="ps")
                for c in range(FC):
                    nc.tensor.matmul(
                        h_p,
                        w1sb[:, c, m * P:(m + 1) * P],
                        qTsb[:, c, n * 512:(n + 1) * 512],
                        start=(c == 0),
                        stop=(c == FC - 1),
                    )
                nc.scalar.activation(
                    hTsb[:, m, n * 512:(n + 1) * 512], h_p, AF.Relu,
                    bias=b1sb[:, m:m + 1], scale=1.0,
                )

        # ---- mlp2: awT = w2^T @ hT + b2  [F, S] ----
        awTsb = big.tile([P, FC, S], f32r, tag="awT")
        for m in range(FC):
            for n in range(NS):
                a_p = ps512.tile([P, 512], f32, tag="ps")
                for c in range(KC):
                    nc.tensor.matmul(
                        a_p,
                        w2sb[:, c, m * P:(m + 1) * P],
                        hTsb[:, c, n * 512:(n + 1) * 512],
                        start=(c == 0),
                        stop=(c == KC - 1),
                    )
                nc.scalar.activation(
                    awTsb[:, m, n * 512:(n + 1) * 512], a_p, AF.Identity,
                    bias=b2sb[:, m:m + 1], scale=1.0,
                )

        # ---- v projection: v_aug = value @ wv_aug + bv_aug  [S, 8*65] bf16
        # (per head: 64 v columns + a ones column from zero-weights + bias=1;
        #  the ones column makes attn@V also emit softmax row sums) ----
        VA = H * (DK + 1)
        vsb = big.tile([P, SC, VA], bf16, tag="v")
        for m in range(SC):
            v_p = ps_sc.tile([P, VA], f32, tag="sc")
            for n0 in (0, 512):
                nw = min(512, VA - n0)
                for c in range(FC):
                    nc.tensor.matmul(
                        v_p[:, n0:n0 + nw],
                        valTsb[:, c, m * P:(m + 1) * P],
                        wvsb[:, c, n0:n0 + nw],
                        start=(c == 0),
                        stop=(c == FC - 1),
                    )
            nc.vector.tensor_add(vsb[:, m, :], v_p, bvsb)

        # ---- per-head: scores -> exp -> attn@V -> xT ----
        xTsb = big.tile([P, FC, S], f32r, tag="qx")
        scale = 1.0 / math.sqrt(DK)

        e_tiles = [None] * H
        rinv_tiles = [None] * H

        def scores_exp(h):
            fc, po = h // 2, (h % 2) * DK
            aw_hT = awTsb[po:po + DK, fc, :]
            e_sb = epool.tile([P, SC, S], bf16, tag="E")
            e_tiles[h] = e_sb
            for m in range(SC):
                sc_p = ps_sc.tile([P, S], f32, tag="sc")
                for n in range(NS):
                    nc.tensor.matmul(
                        sc_p[:, n * 512:(n + 1) * 512],
                        aw_hT[:, m * P:(m + 1) * P],
                        aw_hT[:, n * 512:(n + 1) * 512],
                        start=True,
                        stop=True,
                    )
                nc.scalar.activation(e_sb[:, m, :], sc_p, AF.Exp, scale=scale)

        def attn_v(h):
            # yT' = [v_h | 1]^T @ E  -> [65, S]; row 64 = softmax row sums r.
            # xT_h = yT[:64] * (1/r broadcast across partitions).
            fc, po = h // 2, (h % 2) * DK
            e_sb = e_tiles[h]
            for n in range(NS):
                yt_p = ps_yt.tile([DK + 1, 512], f32, tag="pt")
                for c in range(SC):
                    nc.tensor.matmul(
                        yt_p,
                        vsb[:, c, h * (DK + 1):(h + 1) * (DK + 1)],
                        e_sb[:, c, n * 512:(n + 1) * 512],
                        start=(c == 0),
                        stop=(c == SC - 1),
                    )
                rrow = rpool.tile([1, 512], f32r, tag="rr")
                with nc.allow_low_precision(reason="f32r rounding of 1/rowsum"):
                    nc.vector.reciprocal(rrow, yt_p[DK:DK + 1, :])
                rb_p = ps_yt.tile([DK + 1, 512], f32, tag="pt")
                nc.tensor.matmul(
                    rb_p[:DK, :], ones64, rrow, start=True, stop=True
                )
                rb_sb = rpool.tile([DK, 512], f32, tag="rb")
                nc.scalar.copy(rb_sb, rb_p[:DK, :])
                nc.vector.tensor_mul(
                    xTsb[po:po + DK, fc, n * 512:(n + 1) * 512],
                    yt_p[:DK, :], rb_sb,
                )

        # software-pipeline heads: scores(h+1) issued before attn_v(h)
        scores_exp(0)
        for h in range(1, H):
            scores_exp(h)
            attn_v(h - 1)
        attn_v(H - 1)

        # ---- final: out = x @ wo + bo  [S, F] ----
        for m in range(SC):
            o_p = ps512.tile([P, 512], f32, tag="ps")
            for c in range(FC):
                nc.tensor.matmul(
                    o_p,
                    xTsb[:, c, m * P:(m + 1) * P],
                    wosb[:, c, :],
                    start=(c == 0),
                    stop=(c == FC - 1),
                )
            o_sb = opool.tile([P, F], f32, tag="o")
            nc.vector.tensor_add(o_sb, o_p, bosb)
            nc.sync.dma_start(out_d[m * P:(m + 1) * P, :], o_sb)


def _get_nc(repeat=1):
    global _CACHED_NC
    if _CACHED_NC is None:
        _CACHED_NC = _build_nc(repeat)
    return _CACHED_NC


def _make_in_maps(inputs):
    query = np.asarray(inputs["query"], np.float32)
    value = np.asarray(inputs["value"], np.float32)
    w1 = np.asarray(inputs["w1"], np.float32)
    b1 = np.asarray(inputs["b1"], np.float32)
    w2 = np.asarray(inputs["w2"], np.float32)
    b2 = np.asarray(inputs["b2"], np.float32)
    wv = np.asarray(inputs["wv"], np.float32)
    bv = np.asarray(inputs["bv"], np.float32)
    wo = np.asarray(inputs["wo"], np.float32)
    bo = np.asarray(inputs["bo"], np.float32)

    b1r = np.ascontiguousarray(b1.reshape(HID // P, P).T)
    b2r = np.ascontiguousarray(b2.reshape(F // P, P).T)
    # augmented V projection: per head 64 wv columns + one zero column whose
    # bias of 1.0 creates a ones column in v_aug (gives softmax row sums)
    VA = H * (DK + 1)
    wv_aug = np.zeros((F, VA), np.float32)
    bv_aug = np.zeros(VA, np.float32)
    for h in range(H):
        wv_aug[:, h * (DK + 1):h * (DK + 1) + DK] = wv[:, h * DK:(h + 1) * DK]
        bv_aug[h * (DK + 1):h * (DK + 1) + DK] = bv[h * DK:(h + 1) * DK]
        bv_aug[h * (DK + 1) + DK] = 1.0
    bvb = np.ascontiguousarray(np.broadcast_to(bv_aug, (P, VA)))
    bob = np.ascontiguousarray(np.broadcast_to(bo, (P, F)))

    shared = dict(w1=w1, w2=w2, wv=wv_aug, wo=wo, b1r=b1r, b2r=b2r, bvb=bvb,
                  bob=bob, ident=np.eye(P, dtype=np.float32),
                  ones64=np.ones((1, DK), np.float32))
    return [dict(q=query[i], v=value[i], **shared) for i in range(N_CORES)]


def kernel(**inputs):
    in_maps = _make_in_maps(inputs)

    from concourse.bass_utils import run_bass_kernel_spmd

    nc = _get_nc()
    res = run_bass_kernel_spmd(nc, in_maps, core_ids=list(range(N_CORES)))
    out = np.stack([res.results[i]["out"] for i in range(N_CORES)], axis=0)
    return out.astype(np.float32)


if __name__ == "__main__":
    nc = _get_nc()
    print("built ok")


# revision 15
# speedup vs baseline: 7111.1591x; 1.1451x over previous
"""Dense Synthesizer Attention — Trainium2 Bass kernel.

Sharding: data-parallel over batch. B=8 batch elements, 8 NeuronCores,
one batch element per core, zero collectives.

Per-core computation (S=1024 tokens, F=512 feat, H=8 heads, dk=64):
    hT  = relu(w1^T @ qT + b1)          [1024, 1024]   (qT via PE transpose)
    awT = w2^T @ hT + b2                [512, 1024]
    per head h: aw_hT = awT[64h:64h+64, :]
      scores_m = aw_hT[:, m-tile].T @ aw_hT         (K=64, fp32r)
      E = exp(scores/8)  bf16; ScalarE accum_out -> row sums r (per-partition)
      yT_h = v_h^T @ E  [64, S]  (bf16; E == E^T since scores symmetric,
             so the E tiles written [q, k] serve directly as [k, q])
    out = sum_h (yT_h^T @ wo_h) * (1/r_h)[q] + bo   (per-head K=64 partials
          scaled per-partition by DVE scalar_tensor_tensor, softmax division
          fused into the output projection)

All dims are multiples of 128; everything stays on-chip between stages.
"""

import math

import numpy as np

B, S, F = 8, 1024, 512
H, DK = 8, 64
HID = 2 * F
P = 128

N_CORES = 8

_CACHED_NC = None


def _build_nc(repeat=1):
    from contextlib import ExitStack

    import concourse.mybir as mybir
    import concourse.tile as tile
    from concourse import bacc

    dt = mybir.dt
    f32, f32r = dt.float32, dt.float32r

    SC = S // P      # 8 token chunks
    FC = F // P      # 4 feature chunks
    KC = HID // P    # 8 hidden chunks

    nc = bacc.Bacc(
        "TRN2",
        target_bir_lowering=False,
        debug=False,
        num_devices=N_CORES,
    )

    q_d = nc.declare_dram_parameter("q", [S, F], f32, isOutput=False)
    v_d = nc.declare_dram_parameter("v", [S, F], f32, isOutput=False)
    w1_d = nc.declare_dram_parameter("w1", [F, HID], f32, isOutput=False)
    w2_d = nc.declare_dram_parameter("w2", [HID, F], f32, isOutput=False)
    wv_d = nc.declare_dram_parameter("wv", [F, F], f32, isOutput=False)
    wo_d = nc.declare_dram_parameter("wo", [F, F], f32, isOutput=False)
    b1_d = nc.declare_dram_parameter("b1r", [P, KC], f32, isOutput=False)
    b2_d = nc.declare_dram_parameter("b2r", [P, FC], f32, isOutput=False)
    bv_d = nc.declare_dram_parameter("bvb", [P, F], f32, isOutput=False)
    bo_d = nc.declare_dram_parameter("bob", [P, F], f32, isOutput=False)
    id_d = nc.declare_dram_parameter("ident", [P, P], f32, isOutput=False)
    out_d = nc.declare_dram_parameter("out", [S, F], f32, isOutput=True)

    with ExitStack() as ctx:
        tc = ctx.enter_context(tile.TileContext(nc))

        const = ctx.enter_context(tc.tile_pool(name="const", bufs=1))
        ld = ctx.enter_context(tc.tile_pool(name="ld", bufs=3))
        big = ctx.enter_context(tc.tile_pool(name="big", bufs=1))
        # valT + per-head E tiles are the same byte size; share 4 slots
        sh16 = ctx.enter_context(tc.tile_pool(name="sh16", bufs=3))
        rpool = ctx.enter_context(tc.tile_pool(name="rpool", bufs=1))
        opool = ctx.enter_context(tc.tile_pool(name="opool", bufs=3))

        ps512 = ctx.enter_context(tc.tile_pool(name="ps512", bufs=2, space="PSUM"))
        ps_sc = ctx.enter_context(tc.tile_pool(name="ps_sc", bufs=2, space="PSUM"))
        ps_yt = ctx.enter_context(tc.tile_pool(name="ps_yt", bufs=2, space="PSUM"))

        # ---- constants ----
        ident = const.tile([P, P], f32r)
        nc.sync.dma_start(ident, id_d[:, :].bitcast(f32r))

        w1sb = const.tile([P, FC, HID], f32r)
        nc.sync.dma_start(w1sb, w1_d.rearrange("(c p) k -> p c k", p=P).bitcast(f32r))
        w2sb = const.tile([P, KC, F], f32r)
        nc.sync.dma_start(w2sb, w2_d.rearrange("(c p) f -> p c f", p=P).bitcast(f32r))
        wvsb = const.tile([P, FC, F], f32r)
        nc.sync.dma_start(wvsb, wv_d.rearrange("(c p) f -> p c f", p=P).bitcast(f32r))
        wosb = const.tile([P, FC, F], f32r)
        nc.sync.dma_start(wosb, wo_d.rearrange("(c p) f -> p c f", p=P).bitcast(f32r))
        b1sb = const.tile([P, KC], f32)
        nc.sync.dma_start(b1sb, b1_d[:, :])
        b2sb = const.tile([P, FC], f32)
        nc.sync.dma_start(b2sb, b2_d[:, :])
        bvsb = const.tile([P, F], f32)
        nc.sync.dma_start(bvsb, bv_d[:, :])
        bosb = const.tile([P, F], f32)
        nc.sync.dma_start(bosb, bo_d[:, :])

        consts = (w1sb, w2sb, wvsb, wosb, b1sb, b2sb, bvsb, bosb, ident)
        for _rep in range(repeat):
            _build_body(nc, mybir, ld, big, sh16, rpool, opool,
                        ps512, ps_sc, ps_yt, q_d, v_d, out_d, consts)

    nc.compile()
    return nc


def _build_body(nc, mybir, ld, big, sh16, rpool, opool,
                ps512, ps_sc, ps_yt, q_d, v_d, out_d, consts):
    w1sb, w2sb, wvsb, wosb, b1sb, b2sb, bvsb, bosb, ident = consts
    dt = mybir.dt
    AF = mybir.ActivationFunctionType
    ALU = mybir.AluOpType
    f32, f32r, bf16 = dt.float32, dt.float32r, dt.bfloat16
    SC, FC, KC, NS = S // P, F // P, HID // P, S // 512

    # ---- qT / valT: PE-transpose inputs into [F, S] layout ----
    qTsb = big.tile([P, FC, S], f32r, tag="qx")
    valTsb = sh16.tile([P, FC, S], f32r, tag="sh")
    for src, dstT in ((q_d, qTsb), (v_d, valTsb)):
        for sc in range(SC):
            t_in = ld.tile([P, F], f32r, tag="ld")
            nc.sync.dma_start(t_in, src[sc * P:(sc + 1) * P, :].bitcast(f32r))
            for fc in range(FC):
                t_p = ps_yt.tile([P, P], f32, tag="pt")
                nc.tensor.transpose(
                    t_p.bitcast(f32r), t_in[:, fc * P:(fc + 1) * P], ident
                )
                nc.vector.tensor_copy(dstT[:, fc, sc * P:(sc + 1) * P], t_p)

    # ---- mlp1: hT = relu(w1^T @ qT + b1)  [HID, S] ----
    hTsb = big.tile([P, KC, S], f32r, tag="hT")
    for m in range(KC):
        for n in range(NS):
            h_p = ps512.tile([P, 512], f32, tag="ps")
            for c in range(FC):
                nc.tensor.matmul(
                    h_p,
                    w1sb[:, c, m * P:(m + 1) * P],
                    qTsb[:, c, n * 512:(n + 1) * 512],
                    start=(c == 0),
                    stop=(c == FC - 1),
                )
            nc.scalar.activation(
                hTsb[:, m, n * 512:(n + 1) * 512], h_p, AF.Relu,
                bias=b1sb[:, m:m + 1], scale=1.0,
            )

    # ---- mlp2: awT = w2^T @ hT + b2  [F, S] ----
    awTsb = big.tile([P, FC, S], f32r, tag="awT")
    for m in range(FC):
        for n in range(NS):
            a_p = ps512.tile([P, 512], f32, tag="ps")
            for c in range(KC):
                nc.tensor.matmul(
                    a_p,
                    w2sb[:, c, m * P:(m + 1) * P],
                    hTsb[:, c, n * 512:(n + 1) * 512],
                    start=(c == 0),
                    stop=(c == KC - 1),
                )
            nc.scalar.activation(
                awTsb[:, m, n * 512:(n + 1) * 512], a_p, AF.Identity,
                bias=b2sb[:, m:m + 1], scale=1.0,
            )

    # ---- v projection: v = value @ wv + bv  [S, F] natural, bf16 ----
    vsb = big.tile([P, SC, F], bf16, tag="v")
    for m in range(SC):
        v_p = ps512.tile([P, 512], f32, tag="ps")
        for c in range(FC):
            nc.tensor.matmul(
                v_p,
                valTsb[:, c, m * P:(m + 1) * P],
                wvsb[:, c, :],
                start=(c == 0),
                stop=(c == FC - 1),
            )
        nc.vector.tensor_add(vsb[:, m, :], v_p, bvsb)

    # ---- per-head: scores -> exp(+rowsum) -> yT = v^T @ E ----
    yTsb = big.tile([P, FC, S], f32r, tag="qx")  # reuses qT slot
    scale = 1.0 / math.sqrt(DK)
    e_tiles = [None] * H
    rsum_all = rpool.tile([P, H, SC], f32, tag="rs")
    rinv_all = rpool.tile([P, H, SC], f32, tag="ri")

    def scores_exp(h):
        fc, po = h // 2, (h % 2) * DK
        aw_hT = awTsb[po:po + DK, fc, :]
        e_sb = sh16.tile([P, SC, S], bf16, tag="sh")
        rsum = rsum_all[:, h, :]
        rinv = rinv_all[:, h, :]
        e_tiles[h] = e_sb
        for m in range(SC):
            sc_p = ps_sc.tile([P, S], f32, tag="sc")
            for n in range(NS):
                nc.tensor.matmul(
                    sc_p[:, n * 512:(n + 1) * 512],
                    aw_hT[:, m * P:(m + 1) * P],
                    aw_hT[:, n * 512:(n + 1) * 512],
                    start=True,
                    stop=True,
                )
            nc.scalar.activation(
                e_sb[:, m, :], sc_p, AF.Exp, scale=scale,
                accum_out=rsum[:, m:m + 1],
            )
        nc.vector.reciprocal(rinv, rsum)

    def attn_v(h):
        # yT_h = v_h^T @ E  [64, S] via K=sk accumulation (E symmetric)
        e_sb = e_tiles[h]
        fc, po = h // 2, (h % 2) * DK
        for n in range(NS):
            yt_p = ps_yt.tile([DK, 512], f32, tag="pt")
            for c in range(SC):
                nc.tensor.matmul(
                    yt_p,
                    vsb[:, c, h * DK:(h + 1) * DK],
                    e_sb[:, c, n * 512:(n + 1) * 512],
                    start=(c == 0),
                    stop=(c == SC - 1),
                )
            nc.scalar.copy(yTsb[po:po + DK, fc, n * 512:(n + 1) * 512], yt_p)

    # 2-deep software pipeline over heads to keep PE dense while ACT exps run
    scores_exp(0)
    scores_exp(1)
    for h in range(2, H):
        scores_exp(h)
        attn_v(h - 2)
    attn_v(H - 2)
    attn_v(H - 1)

    # ---- final: out[q] = sum_h (yT_h^T @ wo_h) * rinv_h[q] + bo ----
    for m in range(SC):
        o_sb = opool.tile([P, F], f32, tag="o")
        for h in range(H):
            fc, po = h // 2, (h % 2) * DK
            o_p = ps512.tile([P, 512], f32, tag="ps")
            nc.tensor.matmul(
                o_p,
                yTsb[po:po + DK, fc, m * P:(m + 1) * P],
                wosb[po:po + DK, fc, :],
                start=True,
                stop=True,
            )
            nc.vector.scalar_tensor_tensor(
                o_sb, o_p, rinv_all[:, h, m:m + 1],
                bosb if h == 0 else o_sb,
                ALU.mult, ALU.add,
            )
        nc.sync.dma_start(out_d[m * P:(m + 1) * P, :], o_sb)


def _get_nc(repeat=1):
    global _CACHED_NC
    if _CACHED_NC is None:
        _CACHED_NC = _build_nc(repeat)
    return _CACHED_NC


def _make_in_maps(inputs):
    query = np.asarray(inputs["query"], np.float32)
    value = np.asarray(inputs["value"], np.float32)
    w1 = np.asarray(inputs["w1"], np.float32)
    b1 = np.asarray(inputs["b1"], np.float32)
    w2 = np.asarray(inputs["w2"], np.float32)
    b2 = np.asarray(inputs["b2"], np.float32)
    wv = np.asarray(inputs["wv"], np.float32)
    bv = np.asarray(inputs["bv"], np.float32)
    wo = np.asarray(inputs["wo"], np.float32)
    bo = np.asarray(inputs["bo"], np.float32)

    b1r = np.ascontiguousarray(b1.reshape(HID // P, P).T)
    b2r = np.ascontiguousarray(b2.reshape(F // P, P).T)
    bvb = np.ascontiguousarray(np.broadcast_to(bv, (P, F)))
    bob = np.ascontiguousarray(np.broadcast_to(bo, (P, F)))

    shared = dict(w1=w1, w2=w2, wv=wv, wo=wo, b1r=b1r, b2r=b2r, bvb=bvb,
                  bob=bob, ident=np.eye(P, dtype=np.float32))
    return [dict(q=query[i], v=value[i], **shared) for i in range(N_CORES)]


def kernel(**inputs):
    in_maps = _make_in_maps(inputs)

    from concourse.bass_utils import run_bass_kernel_spmd

    nc = _get_nc()
    res = run_bass_kernel_spmd(nc, in_maps, core_ids=list(range(N_CORES)))
    out = np.stack([res.results[i]["out"] for i in range(N_CORES)], axis=0)
    return out.astype(np.float32)


if __name__ == "__main__":
    nc = _get_nc()
    print("built ok")


# revision 16
# speedup vs baseline: 7546.6364x; 1.0612x over previous
"""Dense Synthesizer Attention — Trainium2 Bass kernel.

Sharding: data-parallel over batch. B=8 batch elements, 8 NeuronCores,
one batch element per core, zero collectives.

Per-core computation (S=1024 tokens, F=512 feat, H=8 heads, dk=64):
    hT  = relu(w1^T @ qT + b1)          [1024, 1024]   (qT via PE transpose)
    awT = w2^T @ hT + b2                [512, 1024]
    per head h: aw_hT = awT[64h:64h+64, :]
      scores_m = aw_hT[:, m-tile].T @ aw_hT         (K=64, fp32r)
      E = exp(scores/8)  bf16; ScalarE accum_out -> row sums r (per-partition)
      yT_h = v_h^T @ E  [64, S]  (bf16; E == E^T since scores symmetric,
             so the E tiles written [q, k] serve directly as [k, q])
    out = sum_h (yT_h^T @ wo_h) * (1/r_h)[q] + bo   (per-head K=64 partials
          scaled per-partition by DVE scalar_tensor_tensor, softmax division
          fused into the output projection)

All dims are multiples of 128; everything stays on-chip between stages.
"""

import math

import numpy as np

B, S, F = 8, 1024, 512
H, DK = 8, 64
HID = 2 * F
P = 128

N_CORES = 8

_CACHED_NC = None


def _build_nc(repeat=1):
    from contextlib import ExitStack

    import concourse.mybir as mybir
    import concourse.tile as tile
    from concourse import bacc

    dt = mybir.dt
    f32, f32r = dt.float32, dt.float32r

    SC = S // P      # 8 token chunks
    FC = F // P      # 4 feature chunks
    KC = HID // P    # 8 hidden chunks

    nc = bacc.Bacc(
        "TRN2",
        target_bir_lowering=False,
        debug=False,
        num_devices=N_CORES,
    )

    q_d = nc.declare_dram_parameter("q", [S, F], f32, isOutput=False)
    v_d = nc.declare_dram_parameter("v", [S, F], f32, isOutput=False)
    w1_d = nc.declare_dram_parameter("w1", [F, HID], dt.bfloat16, isOutput=False)
    w2_d = nc.declare_dram_parameter("w2", [HID, F], dt.bfloat16, isOutput=False)
    wv_d = nc.declare_dram_parameter("wv", [F, F], dt.bfloat16, isOutput=False)
    wo_d = nc.declare_dram_parameter("wo", [F, F], dt.bfloat16, isOutput=False)
    b1_d = nc.declare_dram_parameter("b1r", [P, KC], f32, isOutput=False)
    b2_d = nc.declare_dram_parameter("b2r", [P, FC], f32, isOutput=False)
    bv_d = nc.declare_dram_parameter("bvb", [P, F], f32, isOutput=False)
    bo_d = nc.declare_dram_parameter("bob", [P, F], f32, isOutput=False)
    id_d = nc.declare_dram_parameter("ident", [P, P], f32, isOutput=False)
    out_d = nc.declare_dram_parameter("out", [S, F], f32, isOutput=True)

    with ExitStack() as ctx:
        tc = ctx.enter_context(tile.TileContext(nc))

        const = ctx.enter_context(tc.tile_pool(name="const", bufs=1))
        ld = ctx.enter_context(tc.tile_pool(name="ld", bufs=3))
        big = ctx.enter_context(tc.tile_pool(name="big", bufs=1))
        # valT + per-head E tiles are the same byte size; share 4 slots
        sh16 = ctx.enter_context(tc.tile_pool(name="sh16", bufs=3))
        rpool = ctx.enter_context(tc.tile_pool(name="rpool", bufs=1))
        opool = ctx.enter_context(tc.tile_pool(name="opool", bufs=3))

        ps512 = ctx.enter_context(tc.tile_pool(name="ps512", bufs=2, space="PSUM"))
        ps_sc = ctx.enter_context(tc.tile_pool(name="ps_sc", bufs=2, space="PSUM"))
        ps_yt = ctx.enter_context(tc.tile_pool(name="ps_yt", bufs=2, space="PSUM"))

        # ---- constants ----
        ident = const.tile([P, P], f32r)
        nc.sync.dma_start(ident, id_d[:, :].bitcast(f32r))

        bf16 = dt.bfloat16
        w1sb = const.tile([P, FC, HID], bf16)
        nc.scalar.dma_start(w1sb, w1_d.rearrange("(c p) k -> p c k", p=P))
        w2sb = const.tile([P, KC, F], bf16)
        nc.scalar.dma_start(w2sb, w2_d.rearrange("(c p) f -> p c f", p=P))
        wvsb = const.tile([P, FC, F], bf16)
        nc.scalar.dma_start(wvsb, wv_d.rearrange("(c p) f -> p c f", p=P))
        wosb = const.tile([P, FC, F], bf16)
        nc.scalar.dma_start(wosb, wo_d.rearrange("(c p) f -> p c f", p=P))
        b1sb = const.tile([P, KC], f32)
        nc.scalar.dma_start(b1sb, b1_d[:, :])
        b2sb = const.tile([P, FC], f32)
        nc.scalar.dma_start(b2sb, b2_d[:, :])
        bvsb = const.tile([P, F], f32)
        nc.scalar.dma_start(bvsb, bv_d[:, :])
        bosb = const.tile([P, F], f32)
        nc.scalar.dma_start(bosb, bo_d[:, :])

        consts = (w1sb, w2sb, wvsb, wosb, b1sb, b2sb, bvsb, bosb, ident)
        for _rep in range(repeat):
            _build_body(nc, mybir, ld, big, sh16, rpool, opool,
                        ps512, ps_sc, ps_yt, q_d, v_d, out_d, consts)

    nc.compile()
    return nc


def _build_body(nc, mybir, ld, big, sh16, rpool, opool,
                ps512, ps_sc, ps_yt, q_d, v_d, out_d, consts):
    w1sb, w2sb, wvsb, wosb, b1sb, b2sb, bvsb, bosb, ident = consts
    dt = mybir.dt
    AF = mybir.ActivationFunctionType
    ALU = mybir.AluOpType
    f32, f32r, bf16 = dt.float32, dt.float32r, dt.bfloat16
    SC, FC, KC, NS = S // P, F // P, HID // P, S // 512

    # ---- qT / valT: PE-transpose inputs into [F, S] layout ----
    qTsb = big.tile([P, FC, S], bf16, tag="qx")
    valTsb = sh16.tile([P, FC, S], bf16, tag="sh")
    for src, dstT in ((q_d, qTsb), (v_d, valTsb)):
        for sc in range(SC):
            t_in = ld.tile([P, F], f32r, tag="ld")
            nc.sync.dma_start(t_in, src[sc * P:(sc + 1) * P, :].bitcast(f32r))
            for fc in range(FC):
                t_p = ps_yt.tile([P, P], f32, tag="pt")
                nc.tensor.transpose(
                    t_p.bitcast(f32r), t_in[:, fc * P:(fc + 1) * P], ident
                )
                nc.vector.tensor_copy(dstT[:, fc, sc * P:(sc + 1) * P], t_p)

    # ---- mlp1: hT = relu(w1^T @ qT + b1)  [HID, S] ----
    hTsb = big.tile([P, KC, S], bf16, tag="hT")
    for m in range(KC):
        for n in range(NS):
            h_p = ps512.tile([P, 512], f32, tag="ps")
            for c in range(FC):
                nc.tensor.matmul(
                    h_p,
                    w1sb[:, c, m * P:(m + 1) * P],
                    qTsb[:, c, n * 512:(n + 1) * 512],
                    start=(c == 0),
                    stop=(c == FC - 1),
                )
            nc.scalar.activation(
                hTsb[:, m, n * 512:(n + 1) * 512], h_p, AF.Relu,
                bias=b1sb[:, m:m + 1], scale=1.0,
            )

    # ---- mlp2: awT = w2^T @ hT + b2  [F, S] ----
    awTsb = big.tile([P, FC, S], bf16, tag="awT")
    for m in range(FC):
        for n in range(NS):
            a_p = ps512.tile([P, 512], f32, tag="ps")
            for c in range(KC):
                nc.tensor.matmul(
                    a_p,
                    w2sb[:, c, m * P:(m + 1) * P],
                    hTsb[:, c, n * 512:(n + 1) * 512],
                    start=(c == 0),
                    stop=(c == KC - 1),
                )
            nc.scalar.activation(
                awTsb[:, m, n * 512:(n + 1) * 512], a_p, AF.Identity,
                bias=b2sb[:, m:m + 1], scale=1.0,
            )

    # ---- v projection: v = value @ wv + bv  [S, F] natural, bf16 ----
    vsb = big.tile([P, SC, F], bf16, tag="v")
    for m in range(SC):
        v_p = ps512.tile([P, 512], f32, tag="ps")
        for c in range(FC):
            nc.tensor.matmul(
                v_p,
                valTsb[:, c, m * P:(m + 1) * P],
                wvsb[:, c, :],
                start=(c == 0),
                stop=(c == FC - 1),
            )
        nc.vector.tensor_add(vsb[:, m, :], v_p, bvsb)

    # ---- per-head: scores -> exp(+rowsum) -> yT = v^T @ E ----
    yTsb = big.tile([P, FC, S], bf16, tag="qx")  # reuses qT slot
    scale = 1.0 / math.sqrt(DK)
    e_tiles = [None] * H
    rsum_all = rpool.tile([P, H, SC], f32, tag="rs")
    rinv_all = rpool.tile([P, H, SC], f32, tag="ri")

    def scores_exp(h):
        fc, po = h // 2, (h % 2) * DK
        aw_hT = awTsb[po:po + DK, fc, :]
        e_sb = sh16.tile([P, SC, S], bf16, tag="sh")
        rsum = rsum_all[:, h, :]
        rinv = rinv_all[:, h, :]
        e_tiles[h] = e_sb
        for m in range(SC):
            sc_p = ps_sc.tile([P, S], f32, tag="sc")
            for n in range(NS):
                nc.tensor.matmul(
                    sc_p[:, n * 512:(n + 1) * 512],
                    aw_hT[:, m * P:(m + 1) * P],
                    aw_hT[:, n * 512:(n + 1) * 512],
                    start=True,
                    stop=True,
                )
            nc.scalar.activation(
                e_sb[:, m, :], sc_p, AF.Exp, scale=scale,
                accum_out=rsum[:, m:m + 1],
            )
        nc.vector.reciprocal(rinv, rsum)

    def attn_v(h):
        # yT_h = v_h^T @ E  [64, S] via K=sk accumulation (E symmetric)
        e_sb = e_tiles[h]
        fc, po = h // 2, (h % 2) * DK
        for n in range(NS):
            yt_p = ps_yt.tile([DK, 512], f32, tag="pt")
            for c in range(SC):
                nc.tensor.matmul(
                    yt_p,
                    vsb[:, c, h * DK:(h + 1) * DK],
                    e_sb[:, c, n * 512:(n + 1) * 512],
                    start=(c == 0),
                    stop=(c == SC - 1),
                )
            nc.scalar.copy(yTsb[po:po + DK, fc, n * 512:(n + 1) * 512], yt_p)

    # 2-deep software pipeline over heads to keep PE dense while ACT exps run
    scores_exp(0)
    scores_exp(1)
    for h in range(2, H):
        scores_exp(h)
        attn_v(h - 2)
    attn_v(H - 2)
    attn_v(H - 1)

    # ---- final: out[q] = sum_h (yT_h^T @ wo_h) * rinv_h[q] + bo ----
    for m in range(SC):
        o_sb = opool.tile([P, F], f32, tag="o")
        for h in range(H):
            fc, po = h // 2, (h % 2) * DK
            o_p = ps512.tile([P, 512], f32, tag="ps")
            nc.tensor.matmul(
                o_p,
                yTsb[po:po + DK, fc, m * P:(m + 1) * P],
                wosb[po:po + DK, fc, :],
                start=True,
                stop=True,
            )
            nc.vector.scalar_tensor_tensor(
                o_sb, o_p, rinv_all[:, h, m:m + 1],
                bosb if h == 0 else o_sb,
                ALU.mult, ALU.add,
            )
        nc.sync.dma_start(out_d[m * P:(m + 1) * P, :], o_sb)


def _get_nc(repeat=1):
    global _CACHED_NC
    if _CACHED_NC is None:
        _CACHED_NC = _build_nc(repeat)
    return _CACHED_NC


def _make_in_maps(inputs):
    query = np.asarray(inputs["query"], np.float32)
    value = np.asarray(inputs["value"], np.float32)
    import ml_dtypes
    bf = ml_dtypes.bfloat16
    w1 = np.asarray(inputs["w1"], np.float32)
    b1 = np.asarray(inputs["b1"], np.float32)
    w2 = np.asarray(inputs["w2"], np.float32)
    b2 = np.asarray(inputs["b2"], np.float32)
    wv = np.asarray(inputs["wv"], np.float32)
    bv = np.asarray(inputs["bv"], np.float32)
    wo = np.asarray(inputs["wo"], np.float32)
    bo = np.asarray(inputs["bo"], np.float32)

    b1r = np.ascontiguousarray(b1.reshape(HID // P, P).T)
    b2r = np.ascontiguousarray(b2.reshape(F // P, P).T)
    bvb = np.ascontiguousarray(np.broadcast_to(bv, (P, F)))
    bob = np.ascontiguousarray(np.broadcast_to(bo, (P, F)))

    shared = dict(w1=w1.astype(bf), w2=w2.astype(bf), wv=wv.astype(bf),
                  wo=wo.astype(bf), b1r=b1r, b2r=b2r, bvb=bvb,
                  bob=bob, ident=np.eye(P, dtype=np.float32))
    return [dict(q=query[i], v=value[i], **shared) for i in range(N_CORES)]


def kernel(**inputs):
    in_maps = _make_in_maps(inputs)

    from concourse.bass_utils import run_bass_kernel_spmd

    nc = _get_nc()
    res = run_bass_kernel_spmd(nc, in_maps, core_ids=list(range(N_CORES)))
    out = np.stack([res.results[i]["out"] for i in range(N_CORES)], axis=0)
    return out.astype(np.float32)


if __name__ == "__main__":
    nc = _get_nc()
    print("built ok")


# revision 17
# speedup vs baseline: 8570.4384x; 1.1357x over previous
"""Dense Synthesizer Attention — Trainium2 Bass kernel.

Sharding: data-parallel over batch. B=8 batch elements, 8 NeuronCores,
one batch element per core, zero collectives.

Per-core computation (S=1024 tokens, F=512 feat, H=8 heads, dk=64):
    hT  = relu(w1^T @ qT + b1)          [1024, 1024]   (qT via PE transpose)
    awT = w2^T @ hT + b2                [512, 1024]
    per head h: aw_hT = awT[64h:64h+64, :]
      scores_m = aw_hT[:, m-tile].T @ aw_hT         (K=64, fp32r)
      E = exp(scores/8)  bf16; ScalarE accum_out -> row sums r (per-partition)
      yT_h = v_h^T @ E  [64, S]  (bf16; E == E^T since scores symmetric,
             so the E tiles written [q, k] serve directly as [k, q])
    out = sum_h (yT_h^T @ wo_h) * (1/r_h)[q] + bo   (per-head K=64 partials
          scaled per-partition by DVE scalar_tensor_tensor, softmax division
          fused into the output projection)

All dims are multiples of 128; everything stays on-chip between stages.
"""

import math

import numpy as np

B, S, F = 8, 1024, 512
H, DK = 8, 64
HID = 2 * F
P = 128

N_CORES = 8

_CACHED_NC = None


def _build_nc(repeat=1):
    from contextlib import ExitStack

    import concourse.mybir as mybir
    import concourse.tile as tile
    from concourse import bacc

    dt = mybir.dt
    f32, f32r = dt.float32, dt.float32r

    SC = S // P      # 8 token chunks
    FC = F // P      # 4 feature chunks
    KC = HID // P    # 8 hidden chunks

    nc = bacc.Bacc(
        "TRN2",
        target_bir_lowering=False,
        debug=False,
        num_devices=N_CORES,
    )

    q_d = nc.declare_dram_parameter("q", [S, F], f32, isOutput=False)
    v_d = nc.declare_dram_parameter("v", [S, F], f32, isOutput=False)
    w1_d = nc.declare_dram_parameter("w1", [F, HID], dt.bfloat16, isOutput=False)
    w2_d = nc.declare_dram_parameter("w2", [HID, F], dt.bfloat16, isOutput=False)
    wv_d = nc.declare_dram_parameter("wv", [F, F], dt.bfloat16, isOutput=False)
    wo_d = nc.declare_dram_parameter("wo", [F, F], dt.bfloat16, isOutput=False)
    b1_d = nc.declare_dram_parameter("b1r", [P, KC], f32, isOutput=False)
    b2_d = nc.declare_dram_parameter("b2r", [P, FC], f32, isOutput=False)
    bv_d = nc.declare_dram_parameter("bvb", [P, F], f32, isOutput=False)
    bo_d = nc.declare_dram_parameter("bob", [P, F], f32, isOutput=False)
    id_d = nc.declare_dram_parameter("ident", [P, P], f32, isOutput=False)
    out_d = nc.declare_dram_parameter("out", [S, F], f32, isOutput=True)

    with ExitStack() as ctx:
        tc = ctx.enter_context(tile.TileContext(nc))

        const = ctx.enter_context(tc.tile_pool(name="const", bufs=1))
        ld = ctx.enter_context(tc.tile_pool(name="ld", bufs=3))
        big = ctx.enter_context(tc.tile_pool(name="big", bufs=1))
        # valT + per-head E tiles are the same byte size; share 4 slots
        sh16 = ctx.enter_context(tc.tile_pool(name="sh16", bufs=3))
        rpool = ctx.enter_context(tc.tile_pool(name="rpool", bufs=1))
        opool = ctx.enter_context(tc.tile_pool(name="opool", bufs=1))

        ps512 = ctx.enter_context(tc.tile_pool(name="ps512", bufs=2, space="PSUM"))
        ps_sc = ctx.enter_context(tc.tile_pool(name="ps_sc", bufs=2, space="PSUM"))
        ps_yt = ctx.enter_context(tc.tile_pool(name="ps_yt", bufs=2, space="PSUM"))

        # ---- constants ----
        ident = const.tile([P, P], f32r)
        nc.sync.dma_start(ident, id_d[:, :].bitcast(f32r))

        bf16 = dt.bfloat16
        w1sb = const.tile([P, FC, HID], bf16)
        nc.scalar.dma_start(w1sb, w1_d.rearrange("(c p) k -> p c k", p=P))
        w2sb = const.tile([P, KC, F], bf16)
        nc.scalar.dma_start(w2sb, w2_d.rearrange("(c p) f -> p c f", p=P))
        wvsb = const.tile([P, FC, F], bf16)
        nc.scalar.dma_start(wvsb, wv_d.rearrange("(c p) f -> p c f", p=P))
        wosb = const.tile([P, FC, F], bf16)
        nc.scalar.dma_start(wosb, wo_d.rearrange("(c p) f -> p c f", p=P))
        b1sb = const.tile([P, KC], f32)
        nc.scalar.dma_start(b1sb, b1_d[:, :])
        b2sb = const.tile([P, FC], f32)
        nc.scalar.dma_start(b2sb, b2_d[:, :])
        bvsb = const.tile([P, F], f32)
        nc.scalar.dma_start(bvsb, bv_d[:, :])
        bosb = const.tile([P, F], f32)
        nc.scalar.dma_start(bosb, bo_d[:, :])

        consts = (w1sb, w2sb, wvsb, wosb, b1sb, b2sb, bvsb, bosb, ident)
        for _rep in range(repeat):
            _build_body(nc, mybir, ld, big, sh16, rpool, opool,
                        ps512, ps_sc, ps_yt, q_d, v_d, out_d, consts)

    nc.compile()
    return nc


def _build_body(nc, mybir, ld, big, sh16, rpool, opool,
                ps512, ps_sc, ps_yt, q_d, v_d, out_d, consts):
    w1sb, w2sb, wvsb, wosb, b1sb, b2sb, bvsb, bosb, ident = consts
    dt = mybir.dt
    AF = mybir.ActivationFunctionType
    ALU = mybir.AluOpType
    f32, f32r, bf16 = dt.float32, dt.float32r, dt.bfloat16
    SC, FC, KC, NS = S // P, F // P, HID // P, S // 512

    # ---- qT / valT: PE-transpose inputs into [F, S] layout ----
    qTsb = big.tile([P, FC, S], bf16, tag="qx")
    valTsb = sh16.tile([P, FC, S], bf16, tag="sh")
    for src, dstT in ((q_d, qTsb), (v_d, valTsb)):
        for sc in range(SC):
            t_in = ld.tile([P, F], f32r, tag="ld")
            nc.sync.dma_start(t_in, src[sc * P:(sc + 1) * P, :].bitcast(f32r))
            for fc in range(FC):
                t_p = ps_yt.tile([P, P], f32, tag="pt")
                nc.tensor.transpose(
                    t_p.bitcast(f32r), t_in[:, fc * P:(fc + 1) * P], ident
                )
                nc.vector.tensor_copy(dstT[:, fc, sc * P:(sc + 1) * P], t_p)

    # ---- mlp1: hT = relu(w1^T @ qT + b1)  [HID, S] ----
    hTsb = big.tile([P, KC, S], bf16, tag="hT")
    for m in range(KC):
        for n in range(NS):
            h_p = ps512.tile([P, 512], f32, tag="ps")
            for c in range(FC):
                nc.tensor.matmul(
                    h_p,
                    w1sb[:, c, m * P:(m + 1) * P],
                    qTsb[:, c, n * 512:(n + 1) * 512],
                    start=(c == 0),
                    stop=(c == FC - 1),
                )
            nc.scalar.activation(
                hTsb[:, m, n * 512:(n + 1) * 512], h_p, AF.Relu,
                bias=b1sb[:, m:m + 1], scale=1.0,
            )

    # ---- mlp2: awT = w2^T @ hT + b2  [F, S] ----
    awTsb = big.tile([P, FC, S], bf16, tag="awT")
    for m in range(FC):
        for n in range(NS):
            a_p = ps512.tile([P, 512], f32, tag="ps")
            for c in range(KC):
                nc.tensor.matmul(
                    a_p,
                    w2sb[:, c, m * P:(m + 1) * P],
                    hTsb[:, c, n * 512:(n + 1) * 512],
                    start=(c == 0),
                    stop=(c == KC - 1),
                )
            nc.scalar.activation(
                awTsb[:, m, n * 512:(n + 1) * 512], a_p, AF.Identity,
                bias=b2sb[:, m:m + 1], scale=1.0,
            )

    # ---- v projection: v = value @ wv + bv  [S, F] natural, bf16 ----
    vsb = big.tile([P, SC, F], bf16, tag="v")
    for m in range(SC):
        v_p = ps512.tile([P, 512], f32, tag="ps")
        for c in range(FC):
            nc.tensor.matmul(
                v_p,
                valTsb[:, c, m * P:(m + 1) * P],
                wvsb[:, c, :],
                start=(c == 0),
                stop=(c == FC - 1),
            )
        nc.vector.tensor_add(vsb[:, m, :], v_p, bvsb)

    # ---- per-head: scores -> exp(+rowsum) -> yT = v^T @ E ----
    yTsb = big.tile([P, FC, S], bf16, tag="qx")  # reuses qT slot
    scale = 1.0 / math.sqrt(DK)
    e_tiles = [None] * H
    rsum_all = rpool.tile([P, H, SC], f32, tag="rs")
    rinv_all = rpool.tile([P, H, SC], f32, tag="ri")

    def scores_exp(h):
        fc, po = h // 2, (h % 2) * DK
        aw_hT = awTsb[po:po + DK, fc, :]
        e_sb = sh16.tile([P, SC, S], bf16, tag="sh")
        rsum = rsum_all[:, h, :]
        rinv = rinv_all[:, h, :]
        e_tiles[h] = e_sb
        for m in range(SC):
            sc_p = ps_sc.tile([P, S], f32, tag="sc")
            for n in range(NS):
                nc.tensor.matmul(
                    sc_p[:, n * 512:(n + 1) * 512],
                    aw_hT[:, m * P:(m + 1) * P],
                    aw_hT[:, n * 512:(n + 1) * 512],
                    start=True,
                    stop=True,
                )
            nc.scalar.activation(
                e_sb[:, m, :], sc_p, AF.Exp, scale=scale,
                accum_out=rsum[:, m:m + 1],
            )
        nc.vector.reciprocal(rinv, rsum)

    def attn_v(h):
        # yT_h = v_h^T @ E  [64, S] via K=sk accumulation (E symmetric)
        e_sb = e_tiles[h]
        fc, po = h // 2, (h % 2) * DK
        for n in range(NS):
            yt_p = ps_yt.tile([DK, 512], f32, tag="pt")
            for c in range(SC):
                nc.tensor.matmul(
                    yt_p,
                    vsb[:, c, h * DK:(h + 1) * DK],
                    e_sb[:, c, n * 512:(n + 1) * 512],
                    start=(c == 0),
                    stop=(c == SC - 1),
                )
            nc.scalar.copy(yTsb[po:po + DK, fc, n * 512:(n + 1) * 512], yt_p)

    # ---- final, incrementally per head: out[q] += (yT_h^T @ wo_h)*rinv_h + bo
    # (fused into the head loop so the PE never idles into a cold tail) ----
    o_all = opool.tile([P, SC, F], f32, tag="o")

    def final_partial(h):
        fc, po = h // 2, (h % 2) * DK
        for m in range(SC):
            o_p = ps512.tile([P, 512], f32, tag="ps")
            nc.tensor.matmul(
                o_p,
                yTsb[po:po + DK, fc, m * P:(m + 1) * P],
                wosb[po:po + DK, fc, :],
                start=True,
                stop=True,
            )
            nc.vector.scalar_tensor_tensor(
                o_all[:, m, :], o_p, rinv_all[:, h, m:m + 1],
                bosb if h == 0 else o_all[:, m, :],
                ALU.mult, ALU.add,
            )
        if h == H - 1:
            for m in range(SC):
                nc.sync.dma_start(out_d[m * P:(m + 1) * P, :], o_all[:, m, :])

    # software pipeline over heads: scores(h) | attn_v(h-2) | final(h-4)
    for h in range(H):
        scores_exp(h)
        if h >= 2:
            attn_v(h - 2)
        if h >= 4:
            final_partial(h - 4)
    attn_v(H - 2)
    final_partial(H - 4)
    attn_v(H - 1)
    final_partial(H - 3)
    final_partial(H - 2)
    final_partial(H - 1)


def _get_nc(repeat=1):
    global _CACHED_NC
    if _CACHED_NC is None:
        _CACHED_NC = _build_nc(repeat)
    return _CACHED_NC


def _make_in_maps(inputs):
    query = np.asarray(inputs["query"], np.float32)
    value = np.asarray(inputs["value"], np.float32)
    import ml_dtypes
    bf = ml_dtypes.bfloat16
    w1 = np.asarray(inputs["w1"], np.float32)
    b1 = np.asarray(inputs["b1"], np.float32)
    w2 = np.asarray(inputs["w2"], np.float32)
    b2 = np.asarray(inputs["b2"], np.float32)
    wv = np.asarray(inputs["wv"], np.float32)
    bv = np.asarray(inputs["bv"], np.float32)
    wo = np.asarray(inputs["wo"], np.float32)
    bo = np.asarray(inputs["bo"], np.float32)

    b1r = np.ascontiguousarray(b1.reshape(HID // P, P).T)
    b2r = np.ascontiguousarray(b2.reshape(F // P, P).T)
    bvb = np.ascontiguousarray(np.broadcast_to(bv, (P, F)))
    bob = np.ascontiguousarray(np.broadcast_to(bo, (P, F)))

    shared = dict(w1=w1.astype(bf), w2=w2.astype(bf), wv=wv.astype(bf),
                  wo=wo.astype(bf), b1r=b1r, b2r=b2r, bvb=bvb,
                  bob=bob, ident=np.eye(P, dtype=np.float32))
    return [dict(q=query[i], v=value[i], **shared) for i in range(N_CORES)]


def kernel(**inputs):
    in_maps = _make_in_maps(inputs)

    from concourse.bass_utils import run_bass_kernel_spmd

    nc = _get_nc()
    res = run_bass_kernel_spmd(nc, in_maps, core_ids=list(range(N_CORES)))
    out = np.stack([res.results[i]["out"] for i in range(N_CORES)], axis=0)
    return out.astype(np.float32)


if __name__ == "__main__":
    nc = _get_nc()
    print("built ok")


# revision 18
# speedup vs baseline: 10427.0957x; 1.2166x over previous
"""Dense Synthesizer Attention — Trainium2 Bass kernel.

Sharding: data-parallel over batch. B=8 batch elements, 8 NeuronCores,
one batch element per core, zero collectives.

Per-core computation (S=1024 tokens, F=512 feat, H=8 heads, dk=64):
    hT  = relu(w1^T @ qT + b1)          [1024, 1024]   (qT via PE transpose)
    awT = w2^T @ hT + b2                [512, 1024]
    per head h: aw_hT = awT[64h:64h+64, :]
      scores_m = aw_hT[:, m-tile].T @ aw_hT         (K=64, fp32r)
      E = exp(scores/8)  bf16; ScalarE accum_out -> row sums r (per-partition)
      yT_h = v_h^T @ E  [64, S]  (bf16; E == E^T since scores symmetric,
             so the E tiles written [q, k] serve directly as [k, q])
    out = sum_h (yT_h^T @ wo_h) * (1/r_h)[q] + bo   (per-head K=64 partials
          scaled per-partition by DVE scalar_tensor_tensor, softmax division
          fused into the output projection)

All dims are multiples of 128; everything stays on-chip between stages.
"""

import math

import numpy as np

B, S, F = 8, 1024, 512
H, DK = 8, 64
HID = 2 * F
P = 128

N_CORES = 8

_CACHED_NC = None


def _build_nc(repeat=1):
    from contextlib import ExitStack

    import concourse.mybir as mybir
    import concourse.tile as tile
    from concourse import bacc

    dt = mybir.dt
    f32, f32r = dt.float32, dt.float32r

    SC = S // P      # 8 token chunks
    FC = F // P      # 4 feature chunks
    KC = HID // P    # 8 hidden chunks

    nc = bacc.Bacc(
        "TRN2",
        target_bir_lowering=False,
        debug=False,
        num_devices=N_CORES,
    )

    q_d = nc.declare_dram_parameter("q", [S, F], f32, isOutput=False)
    v_d = nc.declare_dram_parameter("v", [S, F], f32, isOutput=False)
    w1_d = nc.declare_dram_parameter("w1", [F, HID], dt.bfloat16, isOutput=False)
    w2_d = nc.declare_dram_parameter("w2", [HID, F], dt.bfloat16, isOutput=False)
    wv_d = nc.declare_dram_parameter("wv", [F, F], dt.bfloat16, isOutput=False)
    wo_d = nc.declare_dram_parameter("wo", [F, F], dt.bfloat16, isOutput=False)
    b1_d = nc.declare_dram_parameter("b1r", [P, KC], f32, isOutput=False)
    b2_d = nc.declare_dram_parameter("b2r", [P, FC], f32, isOutput=False)
    bv_d = nc.declare_dram_parameter("bvb", [P, F], f32, isOutput=False)
    bo_d = nc.declare_dram_parameter("bob", [P, F], f32, isOutput=False)
    id_d = nc.declare_dram_parameter("ident", [P, P], f32, isOutput=False)
    out_d = nc.declare_dram_parameter("out", [S, F], f32, isOutput=True)

    with ExitStack() as ctx:
        tc = ctx.enter_context(tile.TileContext(nc))

        const = ctx.enter_context(tc.tile_pool(name="const", bufs=1))
        ld = ctx.enter_context(tc.tile_pool(name="ld", bufs=3))
        big = ctx.enter_context(tc.tile_pool(name="big", bufs=1))
        # valT + per-head E tiles are the same byte size; share 4 slots
        sh16 = ctx.enter_context(tc.tile_pool(name="sh16", bufs=3))
        rpool = ctx.enter_context(tc.tile_pool(name="rpool", bufs=1))
        opool = ctx.enter_context(tc.tile_pool(name="opool", bufs=1))

        ps512 = ctx.enter_context(tc.tile_pool(name="ps512", bufs=2, space="PSUM"))
        ps_sc = ctx.enter_context(tc.tile_pool(name="ps_sc", bufs=2, space="PSUM"))
        ps_yt = ctx.enter_context(tc.tile_pool(name="ps_yt", bufs=2, space="PSUM"))

        # ---- constants ----
        ident = const.tile([P, P], f32r)
        nc.sync.dma_start(ident, id_d[:, :].bitcast(f32r))

        bf16 = dt.bfloat16
        w1sb = const.tile([P, FC, HID], bf16)
        nc.scalar.dma_start(w1sb, w1_d.rearrange("(c p) k -> p c k", p=P))
        w2sb = const.tile([P, KC, F], bf16)
        nc.scalar.dma_start(w2sb, w2_d.rearrange("(c p) f -> p c f", p=P))
        wvsb = const.tile([P, FC, F], bf16)
        nc.scalar.dma_start(wvsb, wv_d.rearrange("(c p) f -> p c f", p=P))
        wosb = const.tile([P, FC, F], bf16)
        nc.scalar.dma_start(wosb, wo_d.rearrange("(c p) f -> p c f", p=P))
        b1sb = const.tile([P, KC], f32)
        nc.scalar.dma_start(b1sb, b1_d[:, :])
        b2sb = const.tile([P, FC], f32)
        nc.scalar.dma_start(b2sb, b2_d[:, :])
        bvsb = const.tile([P, F], f32)
        nc.scalar.dma_start(bvsb, bv_d[:, :])
        bosb = const.tile([P, F], f32)
        nc.scalar.dma_start(bosb, bo_d[:, :])

        consts = (w1sb, w2sb, wvsb, wosb, b1sb, b2sb, bvsb, bosb, ident)
        for _rep in range(repeat):
            _build_body(nc, mybir, ld, big, sh16, rpool, opool,
                        ps512, ps_sc, ps_yt, q_d, v_d, out_d, consts)

    nc.compile()
    return nc


def _build_body(nc, mybir, ld, big, sh16, rpool, opool,
                ps512, ps_sc, ps_yt, q_d, v_d, out_d, consts):
    w1sb, w2sb, wvsb, wosb, b1sb, b2sb, bvsb, bosb, ident = consts
    dt = mybir.dt
    AF = mybir.ActivationFunctionType
    ALU = mybir.AluOpType
    f32, f32r, bf16 = dt.float32, dt.float32r, dt.bfloat16
    SC, FC, KC, NS = S // P, F // P, HID // P, S // 512

    # ---- qT / valT: PE-transpose inputs into [F, S] layout ----
    qTsb = big.tile([P, FC, S], bf16, tag="qx")
    valTsb = sh16.tile([P, FC, S], bf16, tag="sh")
    for src, dstT in ((q_d, qTsb), (v_d, valTsb)):
        for sc in range(SC):
            t_in = ld.tile([P, F], f32r, tag="ld")
            nc.sync.dma_start(t_in, src[sc * P:(sc + 1) * P, :].bitcast(f32r))
            for fc in range(FC):
                t_p = ps_yt.tile([P, P], f32, tag="pt")
                nc.tensor.transpose(
                    t_p.bitcast(f32r), t_in[:, fc * P:(fc + 1) * P], ident
                )
                nc.vector.tensor_copy(dstT[:, fc, sc * P:(sc + 1) * P], t_p)

    # ---- mlp1: hT = relu(w1^T @ qT + b1)  [HID, S] ----
    hTsb = big.tile([P, KC, S], bf16, tag="hT")
    for m in range(KC):
        for n in range(NS):
            h_p = ps512.tile([P, 512], f32, tag="ps")
            for c in range(FC):
                nc.tensor.matmul(
                    h_p,
                    w1sb[:, c, m * P:(m + 1) * P],
                    qTsb[:, c, n * 512:(n + 1) * 512],
                    start=(c == 0),
                    stop=(c == FC - 1),
                )
            nc.vector.tensor_scalar(
                hTsb[:, m, n * 512:(n + 1) * 512], h_p,
                b1sb[:, m:m + 1], 0.0, ALU.add, ALU.max,
            )

    # ---- mlp2 (per f-chunk, emitted interleaved with early heads) ----
    awTsb = big.tile([P, FC, S], bf16, tag="awT")

    def mlp2_chunk(m):
        for n in range(NS):
            a_p = ps512.tile([P, 512], f32, tag="ps")
            for c in range(KC):
                nc.tensor.matmul(
                    a_p,
                    w2sb[:, c, m * P:(m + 1) * P],
                    hTsb[:, c, n * 512:(n + 1) * 512],
                    start=(c == 0),
                    stop=(c == KC - 1),
                )
            nc.vector.tensor_scalar_add(
                awTsb[:, m, n * 512:(n + 1) * 512], a_p, b2sb[:, m:m + 1],
            )

    # ---- v projection (per s-chunk, interleaved as well) ----
    vsb = big.tile([P, SC, F], bf16, tag="v")

    def vproj_chunk(m):
        v_p = ps512.tile([P, 512], f32, tag="ps")
        for c in range(FC):
            nc.tensor.matmul(
                v_p,
                valTsb[:, c, m * P:(m + 1) * P],
                wvsb[:, c, :],
                start=(c == 0),
                stop=(c == FC - 1),
            )
        nc.vector.tensor_add(vsb[:, m, :], v_p, bvsb)

    # ---- per-head: scores -> exp(+rowsum) -> yT = v^T @ E ----
    yTsb = big.tile([P, FC, S], bf16, tag="qx")  # reuses qT slot
    scale = 1.0 / math.sqrt(DK)
    e_tiles = [None] * H
    rsum_all = rpool.tile([P, H, SC], f32, tag="rs")
    rinv_all = rpool.tile([P, H, SC], f32, tag="ri")

    def scores_exp(h):
        fc, po = h // 2, (h % 2) * DK
        aw_hT = awTsb[po:po + DK, fc, :]
        e_sb = sh16.tile([P, SC, S], bf16, tag="sh")
        rsum = rsum_all[:, h, :]
        rinv = rinv_all[:, h, :]
        e_tiles[h] = e_sb
        for m in range(SC):
            sc_p = ps_sc.tile([P, S], f32, tag="sc")
            for n in range(NS):
                nc.tensor.matmul(
                    sc_p[:, n * 512:(n + 1) * 512],
                    aw_hT[:, m * P:(m + 1) * P],
                    aw_hT[:, n * 512:(n + 1) * 512],
                    start=True,
                    stop=True,
                )
            nc.scalar.activation(
                e_sb[:, m, :], sc_p, AF.Exp, scale=scale,
                accum_out=rsum[:, m:m + 1],
            )
        nc.vector.reciprocal(rinv, rsum)

    def attn_v(h):
        # yT_h = v_h^T @ E  [64, S] via K=sk accumulation (E symmetric)
        e_sb = e_tiles[h]
        fc, po = h // 2, (h % 2) * DK
        for n in range(NS):
            yt_p = ps_yt.tile([DK, 512], f32, tag="pt")
            for c in range(SC):
                nc.tensor.matmul(
                    yt_p,
                    vsb[:, c, h * DK:(h + 1) * DK],
                    e_sb[:, c, n * 512:(n + 1) * 512],
                    start=(c == 0),
                    stop=(c == SC - 1),
                )
            nc.vector.tensor_copy(yTsb[po:po + DK, fc, n * 512:(n + 1) * 512], yt_p)

    # ---- final, incrementally per head: out[q] += (yT_h^T @ wo_h)*rinv_h + bo
    # (fused into the head loop so the PE never idles into a cold tail) ----
    o_all = opool.tile([P, SC, F], f32, tag="o")

    def final_partial(h):
        fc, po = h // 2, (h % 2) * DK
        for m in range(SC):
            o_p = ps512.tile([P, 512], f32, tag="ps")
            nc.tensor.matmul(
                o_p,
                yTsb[po:po + DK, fc, m * P:(m + 1) * P],
                wosb[po:po + DK, fc, :],
                start=True,
                stop=True,
            )
            nc.vector.scalar_tensor_tensor(
                o_all[:, m, :], o_p, rinv_all[:, h, m:m + 1],
                bosb if h == 0 else o_all[:, m, :],
                ALU.mult, ALU.add,
            )
        if h == H - 1:
            for m in range(SC):
                nc.sync.dma_start(out_d[m * P:(m + 1) * P, :], o_all[:, m, :])

    # software pipeline: mlp2/vproj chunks fill PE while ACT runs exp;
    # then scores(h) | attn_v(h-2) | final(h-4)
    mlp2_chunk(0)
    scores_exp(0)
    mlp2_chunk(1)
    for m in range(SC // 2):
        vproj_chunk(m)
    scores_exp(1)
    mlp2_chunk(2)
    for m in range(SC // 2, SC):
        vproj_chunk(m)
    scores_exp(2)
    attn_v(0)
    mlp2_chunk(3)
    scores_exp(3)
    attn_v(1)
    for h in range(4, H):
        scores_exp(h)
        attn_v(h - 2)
        final_partial(h - 4)
    attn_v(H - 2)
    final_partial(H - 4)
    attn_v(H - 1)
    final_partial(H - 3)
    final_partial(H - 2)
    final_partial(H - 1)


def _get_nc(repeat=1):
    global _CACHED_NC
    if _CACHED_NC is None:
        _CACHED_NC = _build_nc(repeat)
    return _CACHED_NC


def _make_in_maps(inputs):
    query = np.asarray(inputs["query"], np.float32)
    value = np.asarray(inputs["value"], np.float32)
    import ml_dtypes
    bf = ml_dtypes.bfloat16
    w1 = np.asarray(inputs["w1"], np.float32)
    b1 = np.asarray(inputs["b1"], np.float32)
    w2 = np.asarray(inputs["w2"], np.float32)
    b2 = np.asarray(inputs["b2"], np.float32)
    wv = np.asarray(inputs["wv"], np.float32)
    bv = np.asarray(inputs["bv"], np.float32)
    wo = np.asarray(inputs["wo"], np.float32)
    bo = np.asarray(inputs["bo"], np.float32)

    b1r = np.ascontiguousarray(b1.reshape(HID // P, P).T)
    b2r = np.ascontiguousarray(b2.reshape(F // P, P).T)
    bvb = np.ascontiguousarray(np.broadcast_to(bv, (P, F)))
    bob = np.ascontiguousarray(np.broadcast_to(bo, (P, F)))

    shared = dict(w1=w1.astype(bf), w2=w2.astype(bf), wv=wv.astype(bf),
                  wo=wo.astype(bf), b1r=b1r, b2r=b2r, bvb=bvb,
                  bob=bob, ident=np.eye(P, dtype=np.float32))
    return [dict(q=query[i], v=value[i], **shared) for i in range(N_CORES)]


def kernel(**inputs):
    in_maps = _make_in_maps(inputs)

    from concourse.bass_utils import run_bass_kernel_spmd

    nc = _get_nc()
    res = run_bass_kernel_spmd(nc, in_maps, core_ids=list(range(N_CORES)))
    out = np.stack([res.results[i]["out"] for i in range(N_CORES)], axis=0)
    return out.astype(np.float32)


if __name__ == "__main__":
    nc = _get_nc()
    print("built ok")
